# revision 8
# baseline (speedup 1.0000x reference)
"""Two-layer GraphConv (DGL norm='both') on 8 Trainium2 NeuronCores.

kernel(**inputs) takes the full unsharded inputs and returns the full
[2048, 5000] sigmoid output.

Sharding: layer-0 dst nodes (and their edges) are partitioned across the 8
cores; x / W1 / W2 are replicated.  Each core aggregates its 1250 dst rows
with dma_gather + one-hot matmuls, applies W1, and keeps its h shard in its
own HBM.  Layer-1 edges are sharded by src (so each core gathers only from
its local h shard); the partial [2048, 1000] aggregates are combined with a
single ReduceScatter, after which each core computes its 256 output rows
through W2 + sigmoid (transposed so the bias rides the scalar engine).

All floating-point math runs on device.  The host only does integer index
prep: sorting edges, CSR row pointers, padding, and packing.
"""

import sys

if "/opt/trn_rl_repo" not in sys.path:
    sys.path.insert(0, "/opt/trn_rl_repo")

import math

import numpy as np

P = 128
C = 8               # cores
SPLIT0 = 32768      # int16 gather index limit -> split the x table here
GCAP0 = 8           # max 128-row blocks per layer-0 dma_gather
                    # (1024 idxs = the 16 KiB SWDGE descriptor carveout;
                    #  larger single gathers deadlock the Q7 on HW)
GCAP1 = 4           # max blocks per layer-1 dma_gather (4KB rows)


def _ceil(a, b):
    return -(-a // b)


def _wrap_idx16(arr):
    """[n] int -> [128, n//16] int16, wrapped in 16 partitions, replicated 8x."""
    n = arr.shape[0]
    assert n % 16 == 0
    w = arr.astype(np.int16).reshape(n // 16, 16).T
    return np.tile(w, (8, 1)).copy()


def _pack_ptr_pair(sorted_vals, base, rows, cols):
    """rowptr tiles (A, B) f32 [P, cols]: entry (p, q) covers node base + p*cols + q.

    deg(node) = B - A computed on device."""
    ids = base + np.arange(rows * cols).reshape(rows, cols)
    a = np.searchsorted(sorted_vals, ids, side="left")
    b = np.searchsorted(sorted_vals, ids + 1, side="left")
    return a.astype(np.float32), b.astype(np.float32)


def _pack_ptr_win(sorted_vals, base, nwin):
    """rowptr tiles f32 [P, nwin]: entry (p, w) covers node base + w*128 + p."""
    ids = base + (np.arange(nwin)[None, :] * P + np.arange(P)[:, None])
    a = np.searchsorted(sorted_vals, ids, side="left")
    b = np.searchsorted(sorted_vals, ids + 1, side="left")
    return a.astype(np.float32), b.astype(np.float32)


def prep(x, W1, b1, W2, b2, e0_src, e0_dst, e1_src, e1_dst, N1, N2,
         split0=SPLIT0):
    """Host-side integer prep.  Returns (cfg, shared_inputs, per_core_inputs)."""
    x = np.ascontiguousarray(np.asarray(x, dtype=np.float32))
    W1 = np.asarray(W1, dtype=np.float32)
    b1 = np.asarray(b1, dtype=np.float32)
    W2 = np.asarray(W2, dtype=np.float32)
    b2 = np.asarray(b2, dtype=np.float32)
    e0_src = np.asarray(e0_src).astype(np.int64)
    e0_dst = np.asarray(e0_dst).astype(np.int64)
    e1_src = np.asarray(e1_src).astype(np.int64)
    e1_dst = np.asarray(e1_dst).astype(np.int64)

    N0, F = x.shape
    H = W1.shape[1]
    K = W2.shape[1]
    assert (F * 4) % 256 == 0
    S1 = N1 // C
    assert S1 * C == N1
    W0 = _ceil(S1, P)                      # layer-0 dst windows per core
    S2 = N2 // C
    assert S2 * C == N2
    SW2 = _ceil(S2, P)                     # output windows per core
    NW1 = _ceil(N2, P)                     # layer-1 dst windows (global)
    Hp = _ceil(H, P) * P                   # padded h row length
    Kp = Hp                                # contraction pad for W2
    Np = _ceil(K, P) * P
    NCH = Np // P
    KCH = Kp // P
    KF = F // P if F % P == 0 else None
    assert F % P == 0, "feature dim must be a multiple of 128"
    G = 4                                  # output-chunk group size (PSUM banks)
    NG = _ceil(NCH, G)
    NCHp = NG * G

    split0 = min(split0, N0)
    Qlo = _ceil(split0, P)
    n_hi = N0 - split0
    Qhi = max(_ceil(n_hi, P), 1)

    # ---- layer-0 edges: sort by dst, shard by dst range, split lo/hi src ----
    order0 = np.argsort(e0_dst, kind="stable")
    d0 = e0_dst[order0]
    s0 = e0_src[order0]
    core_edges0 = []   # per core: list over sections of (idx16 array, dstoff)
    NLO = np.zeros(W0, dtype=np.int64)
    NHI = np.zeros(W0, dtype=np.int64)
    per_cw = []
    for c in range(C):
        rows = []
        for w in range(W0):
            dlo = c * S1 + w * P
            dhi = min(c * S1 + (w + 1) * P, (c + 1) * S1)
            i0 = np.searchsorted(d0, dlo, side="left")
            i1 = np.searchsorted(d0, dhi, side="left")
            src_w = s0[i0:i1]
            off_w = d0[i0:i1] - dlo
            lo_m = src_w < split0
            rows.append((src_w[lo_m], off_w[lo_m], src_w[~lo_m] - split0,
                         off_w[~lo_m]))
            NLO[w] = max(NLO[w], _ceil(len(src_w[lo_m]), P))
            NHI[w] = max(NHI[w], _ceil(len(src_w[~lo_m]), P))
        per_cw.append(rows)

    # sections: per window, a lo section then a hi section (uniform block
    # counts across cores so the single SPMD NEFF fits every core)
    sections = []          # (w, is_hi, nblocks, idx_col_off, blk_off)
    icol = 0
    iblk = 0
    for w in range(W0):
        for is_hi, nb in ((0, int(NLO[w])), (1, int(NHI[w]))):
            if nb == 0:
                continue
            sections.append((w, is_hi, nb, icol, iblk))
            icol += nb * P // 16
            iblk += nb
    TOTBLK0 = iblk
    TOT0 = TOTBLK0 * P

    g0idx = np.zeros((C, 8 * 16, TOT0 // 16), dtype=np.int16)
    dst0off = np.full((C, P, TOTBLK0), -1.0, dtype=np.float32)
    for c in range(C):
        flat_idx = np.zeros(TOT0, dtype=np.int64)
        flat_off = np.full(TOT0, -1.0, dtype=np.float32)
        for (w, is_hi, nb, icol, iblk) in sections:
            src_l, off_l, src_h, off_h = per_cw[c][w]
            src, off = (src_h, off_h) if is_hi else (src_l, off_l)
            base = iblk * P
            flat_idx[base:base + len(src)] = src
            flat_off[base:base + len(src)] = off
        g0idx[c] = _wrap_idx16(flat_idx)
        dst0off[c] = flat_off.reshape(TOTBLK0, P).T

    # ---- layer-1 edges: shard by src range, sort by dst, global windows ----
    NB1 = np.zeros(NW1, dtype=np.int64)
    per_cj = []
    for c in range(C):
        m = (e1_src >= c * S1) & (e1_src < (c + 1) * S1)
        es = e1_src[m] - c * S1
        ed = e1_dst[m]
        o = np.argsort(ed, kind="stable")
        es, ed = es[o], ed[o]
        rows = []
        for j in range(NW1):
            i0 = np.searchsorted(ed, j * P, side="left")
            i1 = np.searchsorted(ed, min((j + 1) * P, N2), side="left")
            rows.append((es[i0:i1], ed[i0:i1] - j * P))
            NB1[j] = max(NB1[j], _ceil(i1 - i0, P))
        per_cj.append(rows)

    sections1 = []
    icol = 0
    iblk = 0
    for j in range(NW1):
        nb = int(NB1[j])
        if nb == 0:
            continue
        sections1.append((j, nb, icol, iblk))
        icol += nb * P // 16
        iblk += nb
    TOTBLK1 = iblk
    TOT1 = TOTBLK1 * P

    g1idx = np.zeros((C, 8 * 16, max(TOT1 // 16, 1)), dtype=np.int16)
    dst1off = np.full((C, P, max(TOTBLK1, 1)), -1.0, dtype=np.float32)
    for c in range(C):
        flat_idx = np.zeros(max(TOT1, 16), dtype=np.int64)
        flat_off = np.full(max(TOT1, P), -1.0, dtype=np.float32)
        for (j, nb, icol, iblk) in sections1:
            src, off = per_cj[c][j]
            base = iblk * P
            flat_idx[base:base + len(src)] = src
            flat_off[base:base + len(src)] = off
        g1idx[c] = _wrap_idx16(flat_idx[:max(TOT1, 16)])
        dst1off[c] = flat_off[:max(TOTBLK1 * P, P)].reshape(
            max(TOTBLK1, 1), P).T

    # ---- degree rowptr tiles ----
    s0_sorted = np.sort(e0_src)
    p0loA, p0loB = _pack_ptr_pair(s0_sorted, 0, P, Qlo)
    p0hiA, p0hiB = _pack_ptr_pair(s0_sorted, split0, P, Qhi)
    s1_sorted = np.sort(e1_src)
    d1_sorted = np.sort(e1_dst)
    rin0 = []
    ro1 = []
    rin1 = []
    for c in range(C):
        rin0.append(_pack_ptr_win(d0, c * S1, W0))
        ro1.append(_pack_ptr_win(s1_sorted, c * S1, W0))
        rin1.append(_pack_ptr_win(d1_sorted, c * S2, SW2))

    # ---- weights / constants ----
    W2pad = np.zeros((Kp, Np), dtype=np.float32)
    W2pad[:H, :K] = W2
    W2p = np.zeros((NG, KCH, G, P, P), dtype=np.float32)
    for g in range(NG):
        for kc in range(KCH):
            for i in range(G):
                n0 = (g * G + i) * P
                if n0 < Np:
                    W2p[g, kc, i] = W2pad[kc * P:(kc + 1) * P, n0:n0 + P]
    b2pad = np.zeros(NCHp * P, dtype=np.float32)
    b2pad[:K] = b2
    b2p = b2pad.reshape(NCHp, P).T.copy()          # [P, NCHp]
    iota = np.tile(np.arange(P, dtype=np.float32), (P, 1))
    ones_row = np.ones((1, P), dtype=np.float32)
    xhi = np.ascontiguousarray(x[split0:]) if n_hi > 0 else np.zeros(
        (1, F), dtype=np.float32)

    cfg = dict(N0=N0, F=F, H=H, K=K, N1=N1, N2=N2, S1=S1, W0=W0, S2=S2,
               SW2=SW2, NW1=NW1, Hp=Hp, Kp=Kp, Np=Np, NCH=NCH, KCH=KCH,
               NCHp=NCHp, KF=F // P, G=G, NG=NG, Qlo=Qlo, Qhi=Qhi,
               split0=split0, n_hi=n_hi, sections=sections,
               sections1=sections1, TOT0=TOT0, TOTBLK0=TOTBLK0, TOT1=TOT1,
               TOTBLK1=TOTBLK1, NLOmax=int(max(NLO.max(), NHI.max(), 1)),
               NB1max=int(NB1.max() if len(NB1) else 1))

    shared = dict(x=x, xhi=xhi, W1=W1, b1row=b1.reshape(1, H),
                  W2p=W2p, b2p=b2p, iota=iota, ones_row=ones_row,
                  p0loA=p0loA, p0loB=p0loB, p0hiA=p0hiA, p0hiB=p0hiB)
    per_core = []
    for c in range(C):
        per_core.append(dict(
            g0idx=g0idx[c], dst0off=dst0off[c],
            g1idx=g1idx[c], dst1off=dst1off[c],
            rin0A=rin0[c][0], rin0B=rin0[c][1],
            ro1A=ro1[c][0], ro1B=ro1[c][1],
            rin1A=rin1[c][0], rin1B=rin1[c][1]))
    return cfg, shared, per_core


def build_nc(cfg, collective=True):
    """Build the Bass module (one SPMD NEFF for all 8 cores).

    collective=False replaces the ReduceScatter with a local copy (wrong
    numerics across cores, but lets the single-core cost model run)."""
    import concourse.bacc as bacc
    import concourse.bass as bass
    import concourse.mybir as mybir
    import concourse.tile as tile
    from concourse.masks import make_identity

    fp32 = mybir.dt.float32
    i16 = mybir.dt.int16
    Alu = mybir.AluOpType
    Act = mybir.ActivationFunctionType

    F, H, K = cfg["F"], cfg["H"], cfg["K"]
    Hp, Kp = cfg["Hp"], cfg["Kp"]
    W0, SW2, NW1 = cfg["W0"], cfg["SW2"], cfg["NW1"]
    KF, KCH, G, NG = cfg["KF"], cfg["KCH"], cfg["G"], cfg["NG"]
    Qlo, Qhi = cfg["Qlo"], cfg["Qhi"]
    N0, n_hi, split0 = cfg["N0"], cfg["n_hi"], cfg["split0"]
    sections, sections1 = cfg["sections"], cfg["sections1"]

    nc = bacc.Bacc("TRN2", target_bir_lowering=False, debug=False,
                   enable_asserts=False, num_devices=C)

    def din(name, shape, dt=fp32):
        return nc.dram_tensor(name, list(shape), dt, kind="ExternalInput").ap()

    x_ap = din("x", (N0, F))
    xhi_ap = din("xhi", (max(n_hi, 1), F))
    W1_ap = din("W1", (F, H))
    b1_ap = din("b1row", (1, H))
    W2p_ap = din("W2p", (NG, KCH, G, P, P))
    b2p_ap = din("b2p", (P, cfg["NCHp"]))
    iota_ap = din("iota", (P, P))
    ones_ap = din("ones_row", (1, P))
    p0loA_ap = din("p0loA", (P, Qlo))
    p0loB_ap = din("p0loB", (P, Qlo))
    p0hiA_ap = din("p0hiA", (P, Qhi))
    p0hiB_ap = din("p0hiB", (P, Qhi))
    g0idx_ap = din("g0idx", (P, cfg["TOT0"] // 16), i16)
    dst0off_ap = din("dst0off", (P, cfg["TOTBLK0"]))
    g1idx_ap = din("g1idx", (P, max(cfg["TOT1"] // 16, 1)), i16)
    dst1off_ap = din("dst1off", (P, max(cfg["TOTBLK1"], 1)))
    rin0A_ap = din("rin0A", (P, W0))
    rin0B_ap = din("rin0B", (P, W0))
    ro1A_ap = din("ro1A", (P, W0))
    ro1B_ap = din("ro1B", (P, W0))
    rin1A_ap = din("rin1A", (P, SW2))
    rin1B_ap = din("rin1B", (P, SW2))
    outT_ap = nc.dram_tensor("outT", [cfg["NCHp"], P, SW2 * P], fp32,
                             kind="ExternalOutput").ap()

    NLOmax = cfg["NLOmax"]
    NB1max = cfg["NB1max"]

    with tile.TileContext(nc) as tc:
        with tc.tile_pool(name="const", bufs=1) as cp, \
             tc.tile_pool(name="dram", bufs=1, space="DRAM") as dp:
            # ---------- constants ----------
            ident = cp.tile([P, P], fp32, tag="ident")
            make_identity(nc, ident[:])
            iota_sb = cp.tile([P, P], fp32, tag="iota")
            nc.sync.dma_start(out=iota_sb[:], in_=iota_ap[:])
            GMAX = max(GCAP0, GCAP1)
            iota_rep = cp.tile([P, GMAX, P], fp32, tag="iotar")
            nc.vector.tensor_copy(
                out=iota_rep[:],
                in_=iota_sb[:, None, :].to_broadcast([P, GMAX, P]))
            ones_sb = cp.tile([1, P], fp32, tag="ones")
            nc.sync.dma_start(out=ones_sb[:], in_=ones_ap[:])
            b1_sb = cp.tile([1, H], fp32, tag="b1")
            nc.sync.dma_start(out=b1_sb[:], in_=b1_ap[:])
            W1_sb = cp.tile([P, KF, H], fp32, tag="W1")
            for kf in range(KF):
                nc.sync.dma_start(out=W1_sb[:, kf, :],
                                  in_=W1_ap[kf * P:(kf + 1) * P, :])
            b2p_sb = cp.tile([P, cfg["NCHp"]], fp32, tag="b2p")
            nc.sync.dma_start(out=b2p_sb[:], in_=b2p_ap[:])
            g0idx_sb = cp.tile([P, cfg["TOT0"] // 16], i16, tag="g0i")
            nc.sync.dma_start(out=g0idx_sb[:], in_=g0idx_ap[:])
            dst0_sb = cp.tile([P, cfg["TOTBLK0"]], fp32, tag="d0o")
            nc.sync.dma_start(out=dst0_sb[:], in_=dst0off_ap[:])
            g1idx_sb = cp.tile([P, max(cfg["TOT1"] // 16, 1)], i16, tag="g1i")
            nc.sync.dma_start(out=g1idx_sb[:], in_=g1idx_ap[:])
            dst1_sb = cp.tile([P, max(cfg["TOTBLK1"], 1)], fp32, tag="d1o")
            nc.sync.dma_start(out=dst1_sb[:], in_=dst1off_ap[:])

            # ---------- degree -> rsqrt tiles ----------
            def rsqrt_deg(a_ap, b_ap, cols, tag):
                t = cp.tile([P, cols], fp32, tag=tag)
                ta = cp.tile([P, cols], fp32, tag=tag + "a")
                nc.sync.dma_start(out=ta[:], in_=a_ap[:])
                nc.sync.dma_start(out=t[:], in_=b_ap[:])
                nc.vector.tensor_sub(out=t[:], in0=t[:], in1=ta[:])
                nc.vector.tensor_scalar_max(out=t[:], in0=t[:], scalar1=1.0)
                nc.vector.reciprocal(out=t[:], in_=t[:])
                nc.scalar.sqrt(out=t[:], in_=t[:])
                return t

            r0lo = rsqrt_deg(p0loA_ap, p0loB_ap, Qlo, "r0lo")
            r0hi = rsqrt_deg(p0hiA_ap, p0hiB_ap, Qhi, "r0hi")
            rin0 = rsqrt_deg(rin0A_ap, rin0B_ap, W0, "rin0")
            ro1 = rsqrt_deg(ro1A_ap, ro1B_ap, W0, "ro1")
            rin1 = rsqrt_deg(rin1A_ap, rin1B_ap, SW2, "rin1")

            # ---------- per-edge-scale tables in DRAM (64-wide bcast) ----------
            rr_lo = dp.tile([P * Qlo, 64], fp32, tag="rrlo")
            rr_hi = dp.tile([P * Qhi, 64], fp32, tag="rrhi")
            with tc.tile_pool(name="rrbuild", bufs=2) as rp:
                for src_t, dst_t, q in ((r0lo, rr_lo, Qlo), (r0hi, rr_hi, Qhi)):
                    step = 128
                    for q0 in range(0, q, step):
                        qq = min(step, q - q0)
                        rt = rp.tile([P, step, 64], fp32, tag="rt")
                        nc.vector.tensor_copy(
                            out=rt[:, :qq, :],
                            in_=src_t[:, q0:q0 + qq, None].to_broadcast(
                                [P, qq, 64]))
                        nc.sync.dma_start(
                            out=dst_t[:].rearrange(
                                "(p q) e -> p q e", p=P)[:, q0:q0 + qq, :],
                            in_=rt[:, :qq, :])

            S2 = cfg["S2"]
            assert cfg["N2"] % P == 0
            h_dram = dp.tile([W0 * P, Hp], fp32, tag="hd")
            bounce = dp.tile([cfg["N2"], H], fp32, tag="bounce")
            rsout = dp.tile([S2, H], fp32, tag="rsout")

            # ================= layer 0 =================
            with tc.tile_pool(name="l0g", bufs=3) as gp, \
                 tc.tile_pool(name="l0p", bufs=2, space="PSUM") as pp, \
                 tc.tile_pool(name="l0pt", bufs=2, space="PSUM") as ppt, \
                 tc.tile_pool(name="l0ph", bufs=2, space="PSUM") as pph, \
                 tc.tile_pool(name="l0s", bufs=2) as sp:
                blocks_of_w = [[] for _ in range(W0)]
                for si, (w, is_hi, nb, icol, iblk) in enumerate(sections):
                    blocks_of_w[w].append((si, is_hi, nb, icol, iblk))
                for w in range(W0):
                    psum_w = pp.tile([P, F], fp32, space="PSUM", tag="pw")
                    total_nb = sum(nb for (_, _, nb, _, _) in blocks_of_w[w])
                    done_nb = 0
                    if total_nb == 0:
                        zz = sp.tile([P, F], fp32, tag="hpre")
                        nc.vector.memset(zz[:], 0.0)
                        hpre = zz
                    for (si, is_hi, nb, icol, iblk) in blocks_of_w[w]:
                        tbl = xhi_ap if is_hi else x_ap
                        rrt = rr_hi if is_hi else rr_lo
                        for b0 in range(0, nb, GCAP0):
                            bn = min(GCAP0, nb - b0)
                            ni = bn * P
                            gt = gp.tile([P, GCAP0, F], fp32, tag="g")
                            st = gp.tile([P, GCAP0, 64], fp32, tag="s")
                            oh = gp.tile([P, GCAP0, P], fp32, tag="oh")
                            ic = icol + b0 * P // 16
                            nc.gpsimd.dma_gather(
                                gt[:, :bn, :], tbl[:, :],
                                g0idx_sb[:, ic:ic + ni // 16],
                                ni, ni, F)
                            nc.gpsimd.dma_gather(
                                st[:, :bn, :], rrt[:],
                                g0idx_sb[:, ic:ic + ni // 16],
                                ni, ni, 64)
                            bb = iblk + b0
                            nc.vector.tensor_tensor(
                                out=oh[:, :bn, :],
                                in0=iota_rep[:, :bn, :],
                                in1=dst0_sb[:, bb:bb + bn, None].to_broadcast(
                                    [P, bn, P]),
                                op=Alu.is_equal)
                            nc.vector.tensor_tensor(
                                out=oh[:, :bn, :],
                                in0=oh[:, :bn, :],
                                in1=st[:, :bn, 0:1].to_broadcast([P, bn, P]),
                                op=Alu.mult)
                            for b in range(bn):
                                nc.tensor.matmul(
                                    out=psum_w[:],
                                    lhsT=oh[:, b, :],
                                    rhs=gt[:, b, :],
                                    start=(done_nb == 0),
                                    stop=(done_nb == total_nb - 1))
                                done_nb += 1
                    if total_nb > 0:
                        hpre = sp.tile([P, F], fp32, tag="hpre")
                        nc.vector.tensor_tensor(
                            out=hpre[:], in0=psum_w[:],
                            in1=rin0[:, w:w + 1].to_broadcast([P, F]),
                            op=Alu.mult)
                    # transpose h_pre -> lhsT chunks
                    lt = sp.tile([P, KF, P], fp32, tag="lt")
                    for kf in range(KF):
                        pt = ppt.tile([P, P], fp32, space="PSUM", tag="pt")
                        nc.tensor.transpose(out=pt[:],
                                            in_=hpre[:, kf * P:(kf + 1) * P],
                                            identity=ident[:])
                        nc.vector.tensor_copy(out=lt[:, kf, :], in_=pt[:])
                    # W1 matmul + bias, scale by rsqrt(deg_out1), store h
                    ph = pph.tile([P, H], fp32, space="PSUM", tag="ph")
                    for n0 in range(0, H, 512):
                        n1 = min(n0 + 512, H)
                        for kf in range(KF):
                            nc.tensor.matmul(out=ph[:, n0:n1],
                                             lhsT=lt[:, kf, :],
                                             rhs=W1_sb[:, kf, n0:n1],
                                             start=(kf == 0), stop=False)
                        nc.tensor.matmul(out=ph[:, n0:n1],
                                         lhsT=ones_sb[:],
                                         rhs=b1_sb[:, n0:n1],
                                         start=False, stop=True)
                    hsb = sp.tile([P, Hp], fp32, tag="hsb")
                    if Hp > H:
                        nc.vector.memset(hsb[:, H:], 0.0)
                    nc.vector.tensor_tensor(
                        out=hsb[:, :H], in0=ph[:],
                        in1=ro1[:, w:w + 1].to_broadcast([P, H]),
                        op=Alu.mult)
                    nc.sync.dma_start(out=h_dram[w * P:(w + 1) * P, :],
                                      in_=hsb[:])

            # ================= layer 1 =================
            sec1_of_j = {j: None for j in range(NW1)}
            for (j, nb, icol, iblk) in sections1:
                sec1_of_j[j] = (nb, icol, iblk)
            with tc.tile_pool(name="l1g", bufs=3) as gp1, \
                 tc.tile_pool(name="l1p", bufs=2, space="PSUM") as pp1, \
                 tc.tile_pool(name="l1s", bufs=2) as sp1:
                for j in range(NW1):
                    stage = sp1.tile([P, H], fp32, tag="st1")
                    if sec1_of_j[j] is None:
                        nc.vector.memset(stage[:], 0.0)
                    else:
                        nb, icol, iblk = sec1_of_j[j]
                        psum1 = pp1.tile([P, H], fp32, space="PSUM", tag="p1")
                        done = 0
                        for b0 in range(0, nb, GCAP1):
                            bn = min(GCAP1, nb - b0)
                            ni = bn * P
                            gt = gp1.tile([P, GCAP1, Hp], fp32, tag="g1")
                            oh = gp1.tile([P, GCAP1, P], fp32, tag="oh1")
                            ic = icol + b0 * P // 16
                            nc.gpsimd.dma_gather(
                                gt[:, :bn, :], h_dram[:],
                                g1idx_sb[:, ic:ic + ni // 16],
                                ni, ni, Hp)
                            bb = iblk + b0
                            nc.vector.tensor_tensor(
                                out=oh[:, :bn, :],
                                in0=iota_rep[:, :bn, :],
                                in1=dst1_sb[:, bb:bb + bn, None].to_broadcast(
                                    [P, bn, P]),
                                op=Alu.is_equal)
                            for b in range(bn):
                                for n0 in range(0, H, 512):
                                    n1 = min(n0 + 512, H)
                                    nc.tensor.matmul(
                                        out=psum1[:, n0:n1],
                                        lhsT=oh[:, b, :],
                                        rhs=gt[:, b, n0:n1],
                                        start=(done == 0),
                                        stop=(done == nb - 1))
                                done += 1
                        nc.vector.tensor_copy(out=stage[:], in_=psum1[:])
                    nc.sync.dma_start(out=bounce[j * P:(j + 1) * P, :],
                                      in_=stage[:])

            # ================= reduce-scatter =================
            if collective:
                nc.gpsimd.collective_compute(
                    "ReduceScatter", mybir.AluOpType.add,
                    replica_groups=[list(range(C))],
                    ins=[bounce.opt()], outs=[rsout.opt()])
            else:
                with tc.tile_pool(name="rsfake", bufs=2) as rf:
                    for j2 in range(_ceil(S2, P)):
                        rows = min(S2 - j2 * P, P)
                        ft = rf.tile([P, H], fp32, tag="ft")
                        nc.sync.dma_start(
                            out=ft[:rows],
                            in_=bounce[j2 * P:j2 * P + rows, :])
                        nc.sync.dma_start(
                            out=rsout[j2 * P:j2 * P + rows, :],
                            in_=ft[:rows])

            # ================= final W2 + sigmoid (transposed) =================
            with tc.tile_pool(name="f_s", bufs=1) as fs, \
                 tc.tile_pool(name="f_w", bufs=2) as fw, \
                 tc.tile_pool(name="f_pt", bufs=2, space="PSUM") as fpt, \
                 tc.tile_pool(name="f_po", bufs=G, space="PSUM") as fpo, \
                 tc.tile_pool(name="f_o", bufs=3) as fo:
                aggT = fs.tile([P, KCH, SW2 * P], fp32, tag="aggT")
                for j2 in range(SW2):
                    rows = min(S2 - j2 * P, P)
                    a1 = fs.tile([P, Kp], fp32, tag="a1")
                    if rows < P or Kp > H:
                        nc.vector.memset(a1[:], 0.0)
                    nc.sync.dma_start(out=a1[:rows, :H],
                                      in_=rsout[j2 * P:j2 * P + rows, :])
                    nc.vector.tensor_tensor(
                        out=a1[:rows, :H], in0=a1[:rows, :H],
                        in1=rin1[:rows, j2:j2 + 1].to_broadcast([rows, H]),
                        op=Alu.mult)
                    for kc in range(KCH):
                        pt = fpt.tile([P, P], fp32, space="PSUM", tag="fpt")
                        nc.tensor.transpose(out=pt[:],
                                            in_=a1[:, kc * P:(kc + 1) * P],
                                            identity=ident[:])
                        nc.vector.tensor_copy(
                            out=aggT[:, kc, j2 * P:(j2 + 1) * P], in_=pt[:])
                for g in range(NG):
                    w2t = fw.tile([P, KCH, G, P], fp32, tag="w2t")
                    nc.sync.dma_start(
                        out=w2t[:],
                        in_=W2p_ap[g].rearrange("a c k n -> k a c n"))
                    pos = [fpo.tile([P, SW2 * P], fp32, space="PSUM",
                                    tag="po", name=f"po_{g}_{i}")
                           for i in range(G)]
                    for kc in range(KCH):
                        for i in range(G):
                            nc.tensor.matmul(out=pos[i][:],
                                             lhsT=w2t[:, kc, i, :],
                                             rhs=aggT[:, kc, :],
                                             start=(kc == 0),
                                             stop=(kc == KCH - 1))
                    for i in range(G):
                        nch = g * G + i
                        osb = fo.tile([P, SW2 * P], fp32, tag="osb")
                        nc.scalar.activation(
                            out=osb[:], in_=pos[i][:], func=Act.Sigmoid,
                            bias=b2p_sb[:, nch:nch + 1], scale=1.0)
                        nc.sync.dma_start(out=outT_ap[nch], in_=osb[:])

    nc.compile()
    return nc


_CACHE = {}


def _get_nc(cfg):
    key = (cfg["N0"], cfg["F"], cfg["H"], cfg["K"], cfg["N1"], cfg["N2"],
           tuple(cfg["sections"]), tuple(cfg["sections1"]))
    if key not in _CACHE:
        _CACHE[key] = build_nc(cfg)
    return _CACHE[key]


def kernel(x, W1, b1, W2, b2, e0_src, e0_dst, e1_src, e1_dst):
    from concourse.bass_utils import run_bass_kernel_spmd

    N1, N2 = 10000, 2048
    cfg, shared, per_core = prep(x, W1, b1, W2, b2, e0_src, e0_dst,
                                 e1_src, e1_dst, N1, N2)
    nc = _get_nc(cfg)
    in_maps = [dict(shared, **pc) for pc in per_core]
    res = run_bass_kernel_spmd(nc, in_maps, core_ids=list(range(C)))
    S2, SW2, NCH, K = cfg["S2"], cfg["SW2"], cfg["NCH"], cfg["K"]
    out = np.empty((N2, K), dtype=np.float32)
    for c in range(C):
        oT = res.results[c]["outT"]          # [NCHp, P, SW2*P]
        yc = oT.transpose(2, 0, 1).reshape(SW2 * P, -1)[:S2, :K]
        out[c * S2:(c + 1) * S2] = yc
    return out
